# revision 71
# baseline (speedup 1.0000x reference)
# CondConv2d Trainium2 kernel (v4: 1D Winograd F(2,3) along W).
#
# Math (per sample n=(b,l)):
#   pooled[c]   = mean_{h,w} x[n,c,h,w]
#   allxet      = [p0,p0,p0,p1,p2,p3] temporal window (first frame dup'd twice)
#   calib[c,t]  = conv1d(allxet, tconv_w)[c,t] + tconv_b[c]
#   gate[t]     = conv1d(allxet, fc_w)[0,t] + fc_b
#   scale[n,c]  = calib[c,l] + 1
#   out[n,o]    = conv2d(x[n] * scale[n,:,None,None], weight) + bias[o]*(gate[l]+1)
# (the per-sample weight scale is folded into the input because conv is
#  linear in each input channel)
#
# Conv core = Winograd F(2,3) applied along W only (output pairs
# w=2j,2j+1 from the 4-tap window d=[x(2j-1),x(2j),x(2j+1),x(2j+2)]):
#   V0=d0-d2 V1=d1+d2 V2=d2-d1 V3=d1-d3      (DVE, bf16)
#   U0=w0 U1=(w0+w1+w2)/2 U2=(w0-w1+w2)/2 U3=w2   (host, per kh tap)
#   M[xi][oc,h,j] = sum_{ci,kh} U[xi,kh]^T V[xi](row h+kh-1)  (PE, PSUM)
#   y[2j]  = M0+M1+M2+fb   y[2j+1] = M1-M2-M3+fb
# This cuts PE column-streaming 1.5x vs direct conv (the baseline was
# PE-streaming-bound at ~2.37GHz with zero bubbles).
#
# Inverse transform engine split (DVE ops read at most ONE PSUM operand,
# gpsimd can't read PSUM at all):
#   scalar: m1c=M1        A=M0+fb      Bn=-M3+fb     (ACT, 1 PSUM read each)
#   vector: e=M2+m1c      q=-M2+m1c                  (STT, 1 PSUM read each)
#   gpsimd: even=A+e      odd=q+Bn -> osb bf16       (TT, SBUF only)
#
# Sharding: data-parallel over b: 8 cores x 2 batch entries (8 (b,l)
# samples per core). Weights replicated.
#
# Perf notes (from v1-v4 traces):
#  - x staged parity-major ([2,H,JW]) on the host so every forward-
#    transform DVE op reads contiguous runs.
#  - Per-queue DMA transfers serialize; the queue is ~4 descriptors
#    deep, so a 5th dma_start blocks the issuing engine's stream.
#    x on sync, params on scalar (need-order), smalls + two late U
#    chunks on gpsimd's software queue.
#  - fp32 matmuls lower to LOW/HIGH pairs; calib/gate runs bf16.
#  - fp32 warmup matmuls bridge the HAM clock ramp (cold->half->full
#    takes ~13.5us from first PE activity).
#  - output stored bf16 (host upcasts): halves store traffic, adds
#    <0.4% relative quantization vs the 2e-2 budget.

import numpy as np
from ml_dtypes import bfloat16 as np_bf16


def _install_axon_ntff_shim():
    # This container's `antenv` stub lacks `axon_hooks`, which
    # bass_utils imports unconditionally when trace=True under axon.
    import os
    import sys
    import types

    try:
        import antenv.axon_hooks  # noqa: F401

        return
    except Exception:
        pass
    try:
        import antenv
    except Exception:
        return
    mod = types.ModuleType("antenv.axon_hooks")
    mod._hook = None

    def set_axon_ntff_profile_hook(h):
        mod._hook = h

    def get_axon_ntff_profile_hook():
        return mod._hook

    mod.set_axon_ntff_profile_hook = set_axon_ntff_profile_hook
    mod.get_axon_ntff_profile_hook = get_axon_ntff_profile_hook
    sys.modules["antenv.axon_hooks"] = mod
    antenv.axon_hooks = mod
    try:
        from trn_agent_boot.trn_boot import _ntff_profile_via_ctypes

        so = "/opt/axon/libaxon_pjrt.so"
        if os.path.exists(so):
            mod._hook = _ntff_profile_via_ctypes(so)
    except Exception:
        pass


_install_axon_ntff_shim()

import concourse.bass as bass
import concourse.tile as tile
from concourse import mybir
from concourse.bass_utils import run_bass_kernel_spmd

B, L, CIN, COUT, KS, H, W = 16, 4, 256, 256, 3, 32, 32
NCORES = 8
BS = B // NCORES      # batch entries per core
CC = CIN // 128       # ci chunks
OC = COUT // 128      # co chunks
JW = W // 2           # winograd tiles along W
FP32 = mybir.dt.float32
BF16 = mybir.dt.bfloat16
N_WARM = 3            # fp32 warmup matmuls (~2.1us each at cold clock)
ADD = mybir.AluOpType.add
MULT = mybir.AluOpType.mult
BYPASS = mybir.AluOpType.bypass
IDENT = mybir.ActivationFunctionType.Identity

_last_results = None  # test harness reads exec_time_ns from here
_SPLIT_WAITS = True   # debug_sim disables (race detector can't cope)


def _split_excess_waits(nc):
    # walrus in this toolchain encodes exactly one sem wait per engine
    # instruction (TPB_EVENTS has a single wait slot) and optimize_sems
    # is disabled, so Tile can emit instructions with >1 wait that fail
    # codegen ("Too many sync wait commands").  Split the excess waits
    # into standalone EventSemaphore instructions on the same engine
    # stream immediately before the instruction; in-order issue makes
    # this equivalent.
    n = 0
    f = nc.m.functions[0]
    for bb in f.blocks:
        insts = list(bb.instructions)
        out = []
        changed = False
        for inst in insts:
            si = inst.sync_info
            if si is not None:
                waits = list(si.on_wait)
                if len(waits) > 1:
                    for w in waits[:-1]:
                        n += 1
                        es = mybir.InstEventSemaphore(name=f"ES-SPLIT-{n}")
                        es.engine = inst.engine
                        es.sync_info = mybir.SyncInfo(on_wait=[w], on_update=[])
                        out.append(es)
                    si.on_wait = [waits[-1]]
                    inst.sync_info = si
                    changed = True
            out.append(inst)
        if changed:
            bb.instructions = out
    return n


def build_nc():
    nc = bass.Bass()
    # x parity-major: [..., 2(parity), H, JW]. b0 frames l-major (per-l
    # DMAs for fast start), b1 partition-major per-frame.
    x0_d = nc.dram_tensor("x0", [L, 128, CC, 2, H, JW], BF16,
                          kind="ExternalInput")
    x1_d = nc.dram_tensor("x1", [128, L, CC, 2, H, JW], BF16,
                          kind="ExternalInput")
    # winograd-domain weights U, chunked per (oc, xi) for DMA need-order
    w_d = nc.dram_tensor("w", [128, OC, 4, CC, 3, 128], BF16,
                         kind="ExternalInput")
    # tconv weights with the fc (gate) weights folded in as out-channel CIN
    tcw_d = nc.dram_tensor("tcwfcw", [128, CC, 3, CIN + 1], BF16,
                           kind="ExternalInput")
    # [tb1 (CC) | bias2 (OC) | fcb1 (1) | fcw (CC*3)]
    sm_d = nc.dram_tensor("smalls", [128, CC + OC + 1 + CC * 3], FP32,
                          kind="ExternalInput")
    # partition-major bf16 output, un-permuted + upcast on the host
    out_d = nc.dram_tensor("out", [BS, L, 128, OC, H, JW, 2], BF16,
                           kind="ExternalOutput")

    with tile.TileContext(nc) as tc:
        with (
            tc.tile_pool(name="singles", bufs=1) as singles,
            tc.tile_pool(name="outp", bufs=4) as outp,
            tc.tile_pool(name="vpool", bufs=12) as vpool,
            tc.tile_pool(name="iscr", bufs=2) as iscr,
            tc.tile_pool(name="pp_conv", bufs=7, space="PSUM") as pp_conv,
            tc.tile_pool(name="pp_small", bufs=1, space="PSUM") as pp_small,
        ):
            # ---- persistent tiles ----
            w_sb = singles.tile([128, OC, 4, CC, 3, 128], BF16, tag="w")
            tcw_sb = singles.tile([128, CC, 3, CIN + 1], BF16, tag="tcw")
            sm_sb = singles.tile([128, CC + OC + 1 + CC * 3], FP32,
                                 tag="smalls")
            gacc = singles.tile([128, 2, BS, L], BF16, tag="gacc")
            gtmp = singles.tile([128, BS, L], BF16, tag="gtmp")
            gred = singles.tile([128, BS, L], FP32, tag="gred")
            ones_sb = singles.tile([128, 128], BF16, tag="ones")
            warm_sb = singles.tile([128, 512], FP32, tag="warm")

            allxet = singles.tile([128, CC, BS, L + 2], FP32, tag="allxet")
            allxet_bf = singles.tile([128, CC, BS, L + 2], BF16, tag="allxet_bf")
            s_sb = singles.tile([128, CC, BS, L], FP32, tag="s")
            g_sb = singles.tile([1, BS, L], BF16, tag="g")
            fb_sb = singles.tile([128, BS, L, OC], FP32, tag="fb")

            xr0 = {}
            for l in range(L):
                xr = singles.tile([128, CC, 2, H, JW], BF16, tag=f"xr0_{l}")
                xr0[l] = xr
            xr1 = singles.tile([128, L, CC, 2, H, JW], BF16, tag="xr1")
            pscr = singles.tile([128, H, W], BF16, tag="pool_scratch")
            # x_t rows are 17 wide: odd plane stored shifted +1 with a
            # zero column at j=0, even plane with a zero column at
            # j=16 — the W-pad edges of the winograd transform then
            # fall out of the main strided ops (no per-edge cleanup)
            x_t = {}
            for b in range(BS):
                for l in range(L):
                    for ci in range(CC):
                        xt = singles.tile([128, 2, H, JW + 1], BF16,
                                          tag=f"xt{b}_{l}_{ci}")
                        x_t[(b, l, ci)] = xt
            v_t = {}  # winograd-domain inputs, transient

            def xr_ap(b, l, ci):
                if b == 0:
                    return xr0[l][:, ci]
                return xr1[:, l, ci]

            tb1_ap = lambda oc: sm_sb[:, oc:oc + 1]
            bias_ap = lambda oc: sm_sb[:, CC + oc:CC + oc + 1]
            fcb1_ap = sm_sb[:, CC + OC:CC + OC + 1]
            fcw_ap = lambda ci, k: sm_sb[:, CC + OC + 1 + ci * 3 + k:
                                         CC + OC + 2 + ci * 3 + k]

            # ---- t=0: DMAs spread across queues, tiny vector setup ----
            nc.vector.memset(warm_sb[:], 0.0)
            nc.vector.memset(ones_sb[:], 1.0)
            # one-time zero pad columns of every x_t tile (vector is
            # otherwise idle until the first frame lands)
            for xt in x_t.values():
                nc.vector.memset(xt[:, 1, :, 0:1], 0.0)
                nc.vector.memset(xt[:, 0, :, JW:JW + 1], 0.0)

            nc.sync.dma_start(out=xr0[0][:], in_=x0_d[0])
            nc.sync.dma_start(out=xr0[1][:], in_=x0_d[1])
            nc.sync.dma_start(out=xr0[2][:], in_=x0_d[2])
            nc.sync.dma_start(out=xr0[3][:], in_=x0_d[3])
            # b1 per-frame DMAs (partition-major 4KB lines) so pools /
            # gate / transforms for b1 pipeline instead of cliffing on
            # one 2MB transfer
            for l in range(L):
                nc.sync.dma_start(out=xr1[:, l], in_=x1_d[:, l])

            # gpsimd software queue: tiny params + two late U chunks
            nc.gpsimd.dma_start(out=sm_sb[:], in_=sm_d[:])
            nc.gpsimd.dma_start(out=w_sb[:, 1, 1], in_=w_d[:, 1, 1])
            nc.gpsimd.dma_start(out=w_sb[:, 1, 3], in_=w_d[:, 1, 3])

            # scalar queue in need-order. The 5th+ issues block the
            # scalar stream on earlier transfer completions (~15us),
            # so everything scalar needs to do before then is routed
            # to other engines.
            # tcw first: it heads the conv-start critical path (calib ->
            # scale -> fwd), the first U chunk is only needed ~2us later
            nc.scalar.dma_start(out=tcw_sb[:], in_=tcw_d[:])
            nc.scalar.dma_start(out=w_sb[:, 0, 0], in_=w_d[:, 0, 0])
            # preload the scalar engine's activation table while tcw
            # transfers (the first real ACT otherwise pays ~1.3us)
            nc.scalar.mul(pscr[0:1, 0:1, 0:1], warm_sb[0:1, 0:1], 1.0)
            nc.scalar.dma_start(out=w_sb[:, 0, 1], in_=w_d[:, 0, 1])
            # frame-0 ci1 pool on the scalar engine: runs while the first
            # w chunk transfers, without delaying the w issue chain
            nc.scalar.activation(
                pscr[:], xr0[0][:, 1].rearrange("p t h j -> p (t h) j"),
                mybir.ActivationFunctionType.Copy,
                accum_out=allxet[:, 1, 0, 2:3],
            )
            nc.scalar.dma_start(out=w_sb[:, 0, 2], in_=w_d[:, 0, 2])
            nc.scalar.dma_start(out=w_sb[:, 0, 3], in_=w_d[:, 0, 3])
            nc.scalar.dma_start(out=w_sb[:, 1, 0], in_=w_d[:, 1, 0])
            nc.scalar.dma_start(out=w_sb[:, 1, 2], in_=w_d[:, 1, 2])

            # ---- tensor: warmup matmuls (HAM stays un-throttled) ----
            for _ in range(N_WARM):
                wps = pp_conv.tile([128, H, JW], FP32, tag="conv")
                nc.tensor.matmul(
                    wps[:], lhsT=warm_sb[:, 0:128], rhs=warm_sb[:],
                    start=True, stop=True,
                )

            def pool(b, l, ci, eng="v"):
                src = xr_ap(b, l, ci).rearrange("p t h j -> p (t h) j")
                if eng == "v":
                    nc.vector.reduce_sum(
                        out=allxet[:, ci, b, 2 + l:3 + l],
                        in_=src,
                        axis=mybir.AxisListType.XY,
                    )
                else:
                    # scalar-engine pool: ACT copy with free-dim accumulate
                    nc.scalar.activation(
                        pscr[:], src,
                        mybir.ActivationFunctionType.Copy,
                        accum_out=allxet[:, ci, b, 2 + l:3 + l],
                    )

            def calib(bA, bN, l0, nl):
                # calib for batch entries bA..bA+bN-1, frames l0..l0+nl-1
                for oc in range(CC):
                    pc = pp_small.tile([128, BS, L], FP32, tag="small")
                    mms = [(ci, k) for ci in range(CC) for k in range(3)]
                    for i, (ci, k) in enumerate(mms):
                        nc.tensor.matmul(
                            pc[:, 0:bN, 0:nl],
                            lhsT=tcw_sb[:, ci, k, oc * 128:(oc + 1) * 128],
                            rhs=allxet_bf[:, ci, bA:bA + bN, k + l0:k + l0 + nl],
                            start=(i == 0),
                            stop=(i == len(mms) - 1),
                        )
                    with tc.high_priority():
                        nc.vector.tensor_scalar_add(
                            s_sb[:, oc, bA:bA + bN, l0:l0 + nl],
                            pc[:, 0:bN, 0:nl], tb1_ap(oc),
                        )

            def gate_mac(bA, bN, l0, nl):
                # gate conv1d tap MACs on gpsimd (off the PE stream);
                # result lands in gacc[:,1] bf16 ready for the ones-
                # matmul cross-partition reduction
                bs_ = slice(bA, bA + bN)
                a0 = gacc[:, 0, bs_, l0:l0 + nl]
                a1 = gacc[:, 1, bs_, l0:l0 + nl]
                tmp = gtmp[:, bs_, l0:l0 + nl]
                mms = [(ci, k) for ci in range(CC) for k in range(3)]
                for i, (ci, k) in enumerate(mms):
                    src = allxet_bf[:, ci, bs_, k + l0:k + l0 + nl]
                    dst, prev = (a0, a1) if i % 2 == 0 else (a1, a0)
                    if i == 0:
                        nc.gpsimd.tensor_scalar_mul(dst, src, fcw_ap(ci, k))
                    else:
                        # Pool engine can't run STT: mul into a scratch,
                        # then a plain TT accumulate
                        nc.gpsimd.tensor_scalar_mul(tmp, src, fcw_ap(ci, k))
                        nc.gpsimd.tensor_add(out=dst, in0=prev, in1=tmp)

            def gate(bA, bN, l0, nl, fb_list):
                # cross-partition reduce via one all-ones matmul (the
                # result is replicated on every partition), then
                # + (fc_b+1) and fb = bias * (gate+1)
                pg = pp_small.tile([128, BS, L], FP32, tag="small")
                nc.tensor.matmul(
                    pg[:, 0:bN, 0:nl], lhsT=ones_sb[:],
                    rhs=gacc[:, 1, bA:bA + bN, l0:l0 + nl],
                    start=True, stop=True,
                )
                with tc.high_priority():
                    nc.vector.tensor_scalar_add(
                        gred[:, bA:bA + bN, l0:l0 + nl],
                        pg[:, 0:bN, 0:nl], fcb1_ap)
                for b, l in fb_list:
                    for oc in range(OC):
                        nc.gpsimd.tensor_mul(
                            fb_sb[:, b, l, oc:oc + 1],
                            gred[:, b, l:l + 1],
                            bias_ap(oc),
                        )

            def scale_x(b, l, ci, eng="s"):
                # per-(sample, ci-chunk) channel scale folded into x,
                # writing the zero-padded 17-wide layout (even plane at
                # cols 0..15, odd plane shifted to cols 1..16); the
                # output cast produces the bf16 transform operand
                xt = x_t[(b, l, ci)]
                s_ap = s_sb[:, ci, b, l:l + 1]
                xr = xr_ap(b, l, ci)
                if eng == "v":
                    nc.vector.tensor_scalar_mul(
                        xt[:, 0:1, :, 0:JW], xr[:, 0:1], s_ap)
                    nc.vector.tensor_scalar_mul(
                        xt[:, 1:2, :, 1:JW + 1], xr[:, 1:2], s_ap)
                else:
                    nc.scalar.mul(xt[:, 0:1, :, 0:JW], xr[:, 0:1], s_ap)
                    nc.scalar.mul(xt[:, 1:2, :, 1:JW + 1], xr[:, 1:2], s_ap)

            def fwd(b, l):
                # winograd input transform for one sample (both ci),
                # xi-major so the PE's xi0 group can start after the
                # first ops. x_t layout [128, plane, h, 17] with the
                # zero pads making every op a single full-width TT.
                # V0/V1 on vector, V2/V3 on gpsimd.
                vs = []
                for ci in range(CC):
                    v = vpool.tile([128, 4, H, JW], BF16, tag="v")
                    v_t[(b, l, ci)] = v
                    vs.append((v, x_t[(b, l, ci)]))
                for v, x in vs:  # V0 = d0-d2 = odd[j]-odd[j+1]
                    nc.vector.tensor_sub(
                        out=v[:, 0], in0=x[:, 1, :, 0:JW],
                        in1=x[:, 1, :, 1:JW + 1])
                for v, x in vs:  # V1 = d1+d2 = even[j]+odd[j+1]
                    nc.vector.tensor_add(
                        out=v[:, 1], in0=x[:, 0, :, 0:JW],
                        in1=x[:, 1, :, 1:JW + 1])
                for v, x in vs:  # V2 = d2-d1
                    nc.vector.tensor_sub(
                        out=v[:, 2], in0=x[:, 1, :, 1:JW + 1],
                        in1=x[:, 0, :, 0:JW])
                for v, x in vs:  # V3 = d1-d3 = even[j]-even[j+1]
                    nc.vector.tensor_sub(
                        out=v[:, 3], in0=x[:, 0, :, 0:JW],
                        in1=x[:, 0, :, 1:JW + 1])

            # b0 l0 ci0 pool on vector (ci1 ran on scalar above); cast
            # pooled frame-0 and duplicate it into the two pre-pad window
            # columns directly in bf16. high_priority pins the casts
            # ahead of later pools in the scheduler.
            pool(0, 0, 0)
            with tc.high_priority():
                nc.vector.tensor_copy(
                    allxet_bf[:, :, 0:1, 2:3], allxet[:, :, 0:1, 2:3]
                )
                nc.vector.tensor_copy(
                    allxet_bf[:, :, 0:1, 0:1], allxet_bf[:, :, 0:1, 2:3]
                )
                nc.vector.tensor_copy(
                    allxet_bf[:, :, 0:1, 1:2], allxet_bf[:, :, 0:1, 2:3]
                )

            # frame-0 calib/gate/scale: conv (b0,l0) starts on one frame;
            # scales on vector (scalar is DMA-issue-blocked early on)
            calib(0, 1, 0, 1)
            gate_mac(0, 1, 0, 1)
            gate(0, 1, 0, 1, [(0, 0)])
            scale_x(0, 0, 0, "v")
            scale_x(0, 0, 1, "v")
            fwd(0, 0)

            # frame 1 when it lands; the sim-side hold keeps these pools
            # from being scheduled ahead of the frame-0 chain
            with tc.tile_wait_until(0.0145):
                pool(0, 1, 0)
                pool(0, 1, 1)
            nc.vector.tensor_copy(
                allxet_bf[:, :, 0:1, 3:4], allxet[:, :, 0:1, 3:4]
            )
            calib(0, 1, 1, 1)
            gate_mac(0, 1, 1, 1)
            gate(0, 1, 1, 1, [(0, 1)])
            scale_x(0, 1, 0, "v")
            scale_x(0, 1, 1, "v")
            fwd(0, 1)

            # b0 l2/l3 pools + remaining window cast (frames 3/4 on sync)
            with tc.tile_wait_until(0.018):
                pool(0, 2, 0, "v")
                pool(0, 2, 1, "s")
            with tc.tile_wait_until(0.0215):
                pool(0, 3, 0, "v")
                pool(0, 3, 1, "s")
            nc.vector.tensor_copy(
                allxet_bf[:, :, 0:1, 4:6], allxet[:, :, 0:1, 4:6]
            )
            gate_mac(0, 1, 2, 2)

            # b1 pools track the per-frame x1 DMAs; cast + bf16-side dup
            for l in range(L):
                with tc.tile_wait_until(0.025 + 0.0035 * l):
                    pool(1, l, 0, "v")
                    pool(1, l, 1, "s")
            nc.vector.tensor_copy(
                allxet_bf[:, :, 1:2, 2:6], allxet[:, :, 1:2, 2:6]
            )
            nc.vector.tensor_copy(
                allxet_bf[:, :, 1:2, 0:1], allxet_bf[:, :, 1:2, 2:3]
            )
            nc.vector.tensor_copy(
                allxet_bf[:, :, 1:2, 1:2], allxet_bf[:, :, 1:2, 2:3]
            )
            gate_mac(1, 1, 0, L)

            # ---- conv groups ----
            sample_idx = [0]

            def inverse(b, l, oc, Ms, osb, h0, h1):
                # y_even = M0+M1+M2+fb ; y_odd = M1-M2-M3+fb.
                # scalar drains M0/M1/M3 (1-operand ACT affine each),
                # vector drains M2 twice (STT) and runs the two final
                # SBUF combines (fast TTs) into bf16 osb.
                fb_ap = fb_sb[:, b, l, oc:oc + 1]
                hs = slice(h0, h1)
                n = h1 - h0
                m1c = iscr.tile([128, H, JW], FP32, tag="m1c")
                a_t = iscr.tile([128, H, JW], FP32, tag="a")
                bn_t = iscr.tile([128, H, JW], FP32, tag="bn")
                e_t = iscr.tile([128, H, JW], FP32, tag="e")
                q_t = iscr.tile([128, H, JW], FP32, tag="q")
                # PSUM-freeing drains run at high priority: if they sit
                # behind bulk scale/fwd work the PE stalls on bank WARs
                with tc.high_priority():
                    nc.scalar.copy(m1c[:, 0:n], Ms[1][:, hs])
                    nc.scalar.activation(a_t[:, 0:n], Ms[0][:, hs], IDENT,
                                         bias=fb_ap, scale=1.0)
                    nc.scalar.activation(bn_t[:, 0:n], Ms[3][:, hs], IDENT,
                                         bias=fb_ap, scale=-1.0)
                    nc.vector.scalar_tensor_tensor(
                        out=e_t[:, 0:n], in0=Ms[2][:, hs], scalar=1.0,
                        in1=m1c[:, 0:n], op0=MULT, op1=ADD,
                    )
                    nc.vector.scalar_tensor_tensor(
                        out=q_t[:, 0:n], in0=Ms[2][:, hs], scalar=-1.0,
                        in1=m1c[:, 0:n], op0=MULT, op1=ADD,
                    )
                nc.vector.tensor_add(
                    out=osb[:, oc, hs, :, 0], in0=a_t[:, 0:n], in1=e_t[:, 0:n])
                nc.gpsimd.tensor_add(
                    out=osb[:, oc, hs, :, 1], in0=q_t[:, 0:n], in1=bn_t[:, 0:n])

            def conv_sample(b, l, hooks=None):
                osb = outp.tile([128, OC, H, JW, 2], BF16, tag="osb")
                gidx = 0
                last = sample_idx[0] == BS * L - 1
                for oc in range(OC):
                    Ms = []
                    for xi in range(4):
                        ps = pp_conv.tile([128, H, JW], FP32, tag="conv")
                        seq = [(ci, kh) for ci in range(CC)
                               for kh in (1, 0, 2)]
                        for i, (ci, kh) in enumerate(seq):
                            dh = kh - 1
                            hA = max(0, -dh)
                            hB = min(H, H - dh)
                            nc.tensor.matmul(
                                ps[:, hA:hB, :],
                                lhsT=w_sb[:, oc, xi, ci, kh, :],
                                rhs=v_t[(b, l, ci)][:, xi, hA + dh:hB + dh, :],
                                start=(i == 0),
                                stop=(i == len(seq) - 1),
                            )
                        Ms.append(ps)
                        gidx += 1
                        if hooks and gidx in hooks:
                            hooks[gidx]()
                    if last and oc == OC - 1:
                        # final group: h-quarter split so the drain and
                        # the stores pipeline in quarter-size chunks
                        hq = H // 4
                        for q in range(4):
                            inverse(b, l, oc, Ms, osb, q * hq, (q + 1) * hq)
                            nc.sync.dma_start(
                                out=out_d[b, l, :, oc, q * hq:(q + 1) * hq],
                                in_=osb[:, oc, q * hq:(q + 1) * hq],
                            )
                    else:
                        inverse(b, l, oc, Ms, osb, 0, H)
                        if last:
                            st_eng = nc.sync
                        else:
                            # gpsimd's software DMA is ~2x slower: it gets
                            # only the two earliest samples (sync is still
                            # streaming x in), sync carries the rest
                            st_eng = (nc.gpsimd if sample_idx[0] in (0, 2)
                                      else nc.sync)
                        st_eng.dma_start(
                            out=out_d[b, l, :, oc], in_=osb[:, oc]
                        )
                sample_idx[0] += 1

            def tail_calib():
                # b0 l2/l3 calib (frames landed during conv(0,0))
                calib(0, 1, 2, 2)
                for l in (2, 3):
                    scale_x(0, l, 0, "v")
                    scale_x(0, l, 1, "s")
                fwd(0, 2)
                fwd(0, 3)

            def tail_gate_b0():
                # frames 2/3 of b0 only — the b1 half must wait for the
                # last b1 frame DMA (~35us), which would stall the PE here
                gate(0, 1, 2, 2, [(0, 2), (0, 3)])

            def tail_gate_b1():
                gate(1, 1, 0, L, [(1, l) for l in range(L)])

            def tail_b1(l):
                # b1 calib + transforms, one frame per hook so the
                # scale/fwd bursts never starve the drain chain
                def h():
                    if l == 0:
                        calib(1, 1, 0, L)
                    scale_x(1, l, 0, "s")
                    scale_x(1, l, 1, "s")
                    fwd(1, l)
                return h

            conv_sample(0, 0, hooks={6: tail_calib})
            conv_sample(0, 1, hooks={6: tail_gate_b0})
            conv_sample(0, 2, hooks={2: tail_b1(0), 6: tail_gate_b1,
                                     7: tail_b1(1)})
            conv_sample(0, 3, hooks={2: tail_b1(2), 6: tail_b1(3)})
            for l in range(L):
                conv_sample(1, l)

    if _SPLIT_WAITS:
        _split_excess_waits(nc)
    return nc


def kernel(x, weight, bias, tconv_w, tconv_b, fc_w, fc_b):
    global _last_results
    x = np.asarray(x, dtype=np.float32)
    weight = np.asarray(weight, dtype=np.float32)
    bias = np.asarray(bias, dtype=np.float32)
    tconv_w = np.asarray(tconv_w, dtype=np.float32)
    tconv_b = np.asarray(tconv_b, dtype=np.float32)
    fc_w = np.asarray(fc_w, dtype=np.float32)
    fc_b = np.asarray(fc_b, dtype=np.float32)

    HW = H * W
    # host-side packing (shared across cores); 1/(H*W) pooling norm and
    # the +1 biases folded here. x parity-major: (..., 2, H, JW).
    x_bf = (x.astype(np_bf16)
            .reshape(B, L, CC, 128, H, JW, 2))
    # winograd-domain weight taps U per kh row
    wt = weight.transpose(1, 2, 3, 0)                 # (CIN, kh, kw, COUT)
    U = np.stack([
        wt[:, :, 0],
        (wt[:, :, 0] + wt[:, :, 1] + wt[:, :, 2]) * np.float32(0.5),
        (wt[:, :, 0] - wt[:, :, 1] + wt[:, :, 2]) * np.float32(0.5),
        wt[:, :, 2],
    ], axis=2)                                        # (CIN, kh, xi, COUT)
    w_host = np.ascontiguousarray(
        U.reshape(CC, 128, 3, 4, OC, 128)
        .transpose(1, 4, 3, 0, 2, 5)                  # [ci_lo,OC,xi,CC,kh,oc_lo]
        .astype(np_bf16)
    )
    inv = np.float32(1.0 / HW)
    tcw = (tconv_w * inv).transpose(1, 2, 0)          # (CIN_in, 3, CIN_out)
    fcw = (fc_w[0] * inv)[:, :, None]                 # (CIN_in, 3, 1)
    tcw_host = np.ascontiguousarray(
        np.concatenate([tcw, fcw], axis=2)
        .reshape(CC, 128, 3, CIN + 1)
        .transpose(1, 0, 2, 3)
        .astype(np_bf16)
    )
    fcw_cols = ((fc_w[0] * inv).reshape(CC, 128, 3)
                .transpose(1, 0, 2).reshape(128, CC * 3))
    sm_host = np.ascontiguousarray(np.concatenate([
        tconv_b.reshape(CC, 128).T + np.float32(1.0),
        bias.reshape(OC, 128).T,
        np.full((128, 1), fc_b[0] + 1.0, dtype=np.float32),
        fcw_cols.astype(np.float32),
    ], axis=1))

    nc = build_nc()
    in_maps = []
    for core in range(NCORES):
        xc = x_bf[core * BS:(core + 1) * BS]   # (BS, L, CC, 128, H, JW, 2)
        in_maps.append({
            # -> [L, 128, CC, 2, H, JW]
            "x0": np.ascontiguousarray(xc[0].transpose(0, 2, 1, 5, 3, 4)),
            # -> [128, L, CC, 2, H, JW]
            "x1": np.ascontiguousarray(xc[1].transpose(2, 0, 1, 5, 3, 4)),
            "w": w_host,
            "tcwfcw": tcw_host,
            "smalls": sm_host,
        })
    res = run_bass_kernel_spmd(nc, in_maps, core_ids=list(range(NCORES)))
    _last_results = res
    # out_d is [BS, L, 128, OC, H, JW, 2] partition-major bf16 ->
    # upcast + un-permute on the host
    outs = []
    for r in res.results:
        o = (np.asarray(r["out"]).astype(np.float32)
             .reshape(BS, L, 128, OC, HW).transpose(0, 1, 3, 2, 4))
        outs.append(np.ascontiguousarray(o).reshape(BS * L, COUT, H, W))
    return np.concatenate(outs, axis=0)


# revision 73
# speedup vs baseline: 1.0731x; 1.0731x over previous
# CondConv2d Trainium2 kernel (v4: 1D Winograd F(2,3) along W).
#
# Math (per sample n=(b,l)):
#   pooled[c]   = mean_{h,w} x[n,c,h,w]
#   allxet      = [p0,p0,p0,p1,p2,p3] temporal window (first frame dup'd twice)
#   calib[c,t]  = conv1d(allxet, tconv_w)[c,t] + tconv_b[c]
#   gate[t]     = conv1d(allxet, fc_w)[0,t] + fc_b
#   scale[n,c]  = calib[c,l] + 1
#   out[n,o]    = conv2d(x[n] * scale[n,:,None,None], weight) + bias[o]*(gate[l]+1)
# (the per-sample weight scale is folded into the input because conv is
#  linear in each input channel)
#
# Conv core = Winograd F(2,3) applied along W only (output pairs
# w=2j,2j+1 from the 4-tap window d=[x(2j-1),x(2j),x(2j+1),x(2j+2)]):
#   V0=d0-d2 V1=d1+d2 V2=d2-d1 V3=d1-d3      (DVE, bf16)
#   U0=w0 U1=(w0+w1+w2)/2 U2=(w0-w1+w2)/2 U3=w2   (host, per kh tap)
#   M[xi][oc,h,j] = sum_{ci,kh} U[xi,kh]^T V[xi](row h+kh-1)  (PE, PSUM)
#   y[2j]  = M0+M1+M2+fb   y[2j+1] = M1-M2-M3+fb
# This cuts PE column-streaming 1.5x vs direct conv (the baseline was
# PE-streaming-bound at ~2.37GHz with zero bubbles).
#
# Inverse transform engine split (DVE ops read at most ONE PSUM operand,
# gpsimd can't read PSUM at all):
#   scalar: m1c=M1        A=M0+fb      Bn=-M3+fb     (ACT, 1 PSUM read each)
#   vector: e=M2+m1c      q=-M2+m1c                  (STT, 1 PSUM read each)
#   gpsimd: even=A+e      odd=q+Bn -> osb bf16       (TT, SBUF only)
#
# Sharding: data-parallel over b: 8 cores x 2 batch entries (8 (b,l)
# samples per core). Weights replicated.
#
# Perf notes (from v1-v4 traces):
#  - x staged parity-major ([2,H,JW]) on the host so every forward-
#    transform DVE op reads contiguous runs.
#  - Per-queue DMA transfers serialize; the queue is ~4 descriptors
#    deep, so a 5th dma_start blocks the issuing engine's stream.
#    x on sync, params on scalar (need-order), smalls + two late U
#    chunks on gpsimd's software queue.
#  - fp32 matmuls lower to LOW/HIGH pairs; calib/gate runs bf16.
#  - fp32 warmup matmuls bridge the HAM clock ramp (cold->half->full
#    takes ~13.5us from first PE activity).
#  - output stored bf16 (host upcasts): halves store traffic, adds
#    <0.4% relative quantization vs the 2e-2 budget.

import numpy as np
from ml_dtypes import bfloat16 as np_bf16


def _install_axon_ntff_shim():
    # This container's `antenv` stub lacks `axon_hooks`, which
    # bass_utils imports unconditionally when trace=True under axon.
    import os
    import sys
    import types

    try:
        import antenv.axon_hooks  # noqa: F401

        return
    except Exception:
        pass
    try:
        import antenv
    except Exception:
        return
    mod = types.ModuleType("antenv.axon_hooks")
    mod._hook = None

    def set_axon_ntff_profile_hook(h):
        mod._hook = h

    def get_axon_ntff_profile_hook():
        return mod._hook

    mod.set_axon_ntff_profile_hook = set_axon_ntff_profile_hook
    mod.get_axon_ntff_profile_hook = get_axon_ntff_profile_hook
    sys.modules["antenv.axon_hooks"] = mod
    antenv.axon_hooks = mod
    try:
        from trn_agent_boot.trn_boot import _ntff_profile_via_ctypes

        so = "/opt/axon/libaxon_pjrt.so"
        if os.path.exists(so):
            mod._hook = _ntff_profile_via_ctypes(so)
    except Exception:
        pass


_install_axon_ntff_shim()

import concourse.bass as bass
import concourse.tile as tile
from concourse import mybir
from concourse.bass_utils import run_bass_kernel_spmd

B, L, CIN, COUT, KS, H, W = 16, 4, 256, 256, 3, 32, 32
NCORES = 8
BS = B // NCORES      # batch entries per core
CC = CIN // 128       # ci chunks
OC = COUT // 128      # co chunks
JW = W // 2           # winograd tiles along W
FP32 = mybir.dt.float32
BF16 = mybir.dt.bfloat16
N_WARM = 3            # fp32 warmup matmuls (~2.1us each at cold clock)
ADD = mybir.AluOpType.add
MULT = mybir.AluOpType.mult
BYPASS = mybir.AluOpType.bypass
IDENT = mybir.ActivationFunctionType.Identity

_last_results = None  # test harness reads exec_time_ns from here
_SPLIT_WAITS = True   # debug_sim disables (race detector can't cope)


def _split_excess_waits(nc):
    # walrus in this toolchain encodes exactly one sem wait per engine
    # instruction (TPB_EVENTS has a single wait slot) and optimize_sems
    # is disabled, so Tile can emit instructions with >1 wait that fail
    # codegen ("Too many sync wait commands").  Split the excess waits
    # into standalone EventSemaphore instructions on the same engine
    # stream immediately before the instruction; in-order issue makes
    # this equivalent.
    n = 0
    f = nc.m.functions[0]
    for bb in f.blocks:
        insts = list(bb.instructions)
        out = []
        changed = False
        for inst in insts:
            si = inst.sync_info
            if si is not None:
                waits = list(si.on_wait)
                if len(waits) > 1:
                    for w in waits[:-1]:
                        n += 1
                        es = mybir.InstEventSemaphore(name=f"ES-SPLIT-{n}")
                        es.engine = inst.engine
                        es.sync_info = mybir.SyncInfo(on_wait=[w], on_update=[])
                        out.append(es)
                    si.on_wait = [waits[-1]]
                    inst.sync_info = si
                    changed = True
            out.append(inst)
        if changed:
            bb.instructions = out
    return n


def build_nc():
    nc = bass.Bass()
    # x parity-major: [..., 2(parity), H, JW]. b0 frames l-major (per-l
    # DMAs for fast start), b1 partition-major per-frame.
    x0_d = nc.dram_tensor("x0", [L, 128, CC, 2, H, JW], BF16,
                          kind="ExternalInput")
    x1_d = nc.dram_tensor("x1", [128, L, CC, 2, H, JW], BF16,
                          kind="ExternalInput")
    # winograd-domain weights U, chunked per (oc, xi) for DMA need-order
    w_d = nc.dram_tensor("w", [128, OC, 4, CC, 3, 128], BF16,
                         kind="ExternalInput")
    # tconv weights with the fc (gate) weights folded in as out-channel CIN
    tcw_d = nc.dram_tensor("tcwfcw", [128, CC, 3, CIN + 1], BF16,
                           kind="ExternalInput")
    # [tb1 (CC) | bias2 (OC) | fcb1 (1) | fcw (CC*3)]
    sm_d = nc.dram_tensor("smalls", [128, CC + OC + 1 + CC * 3], FP32,
                          kind="ExternalInput")
    # partition-major bf16 output, un-permuted + upcast on the host
    out_d = nc.dram_tensor("out", [BS, L, 128, OC, H, JW, 2], BF16,
                           kind="ExternalOutput")

    with tile.TileContext(nc) as tc:
        with (
            tc.tile_pool(name="singles", bufs=1) as singles,
            tc.tile_pool(name="outp", bufs=4) as outp,
            tc.tile_pool(name="vpool", bufs=12) as vpool,
            tc.tile_pool(name="iscr", bufs=2) as iscr,
            tc.tile_pool(name="pp_conv", bufs=7, space="PSUM") as pp_conv,
            tc.tile_pool(name="pp_small", bufs=1, space="PSUM") as pp_small,
        ):
            # ---- persistent tiles ----
            w_sb = singles.tile([128, OC, 4, CC, 3, 128], BF16, tag="w")
            tcw_sb = singles.tile([128, CC, 3, CIN + 1], BF16, tag="tcw")
            sm_sb = singles.tile([128, CC + OC + 1 + CC * 3], FP32,
                                 tag="smalls")
            gacc = singles.tile([128, 2, BS, L], BF16, tag="gacc")
            gtmp = singles.tile([128, BS, L], BF16, tag="gtmp")
            gred = singles.tile([128, BS, L], FP32, tag="gred")
            ones_sb = singles.tile([128, 128], BF16, tag="ones")
            warm_sb = singles.tile([128, 512], FP32, tag="warm")

            allxet = singles.tile([128, CC, BS, L + 2], FP32, tag="allxet")
            allxet_bf = singles.tile([128, CC, BS, L + 2], BF16, tag="allxet_bf")
            s_sb = singles.tile([128, CC, BS, L], FP32, tag="s")
            g_sb = singles.tile([1, BS, L], BF16, tag="g")
            fb_sb = singles.tile([128, BS, L, OC], FP32, tag="fb")

            xr0 = {}
            for l in range(L):
                xr = singles.tile([128, CC, 2, H, JW], BF16, tag=f"xr0_{l}")
                xr0[l] = xr
            xr1 = singles.tile([128, L, CC, 2, H, JW], BF16, tag="xr1")
            pscr = singles.tile([128, H, W], BF16, tag="pool_scratch")
            # x_t rows are 17 wide: odd plane stored shifted +1 with a
            # zero column at j=0, even plane with a zero column at
            # j=16 — the W-pad edges of the winograd transform then
            # fall out of the main strided ops (no per-edge cleanup)
            x_t = {}
            for b in range(BS):
                for l in range(L):
                    for ci in range(CC):
                        xt = singles.tile([128, 2, H, JW + 1], BF16,
                                          tag=f"xt{b}_{l}_{ci}")
                        x_t[(b, l, ci)] = xt
            v_t = {}  # winograd-domain inputs, transient

            def xr_ap(b, l, ci):
                if b == 0:
                    return xr0[l][:, ci]
                return xr1[:, l, ci]

            tb1_ap = lambda oc: sm_sb[:, oc:oc + 1]
            bias_ap = lambda oc: sm_sb[:, CC + oc:CC + oc + 1]
            fcb1_ap = sm_sb[:, CC + OC:CC + OC + 1]
            fcw_ap = lambda ci, k: sm_sb[:, CC + OC + 1 + ci * 3 + k:
                                         CC + OC + 2 + ci * 3 + k]

            # ---- t=0: DMAs spread across queues, tiny vector setup ----
            nc.vector.memset(warm_sb[:], 0.0)
            nc.vector.memset(ones_sb[:], 1.0)
            # one-time zero pad columns of every x_t tile (vector is
            # otherwise idle until the first frame lands)
            for xt in x_t.values():
                nc.vector.memset(xt[:, 1, :, 0:1], 0.0)
                nc.vector.memset(xt[:, 0, :, JW:JW + 1], 0.0)

            nc.sync.dma_start(out=xr0[0][:], in_=x0_d[0])
            nc.sync.dma_start(out=xr0[1][:], in_=x0_d[1])
            nc.sync.dma_start(out=xr0[2][:], in_=x0_d[2])
            nc.sync.dma_start(out=xr0[3][:], in_=x0_d[3])
            # b1 per-frame DMAs (partition-major 4KB lines) so pools /
            # gate / transforms for b1 pipeline instead of cliffing on
            # one 2MB transfer
            for l in range(L):
                nc.sync.dma_start(out=xr1[:, l], in_=x1_d[:, l])

            # gpsimd software queue: tiny params + two late U chunks
            nc.gpsimd.dma_start(out=sm_sb[:], in_=sm_d[:])
            nc.gpsimd.dma_start(out=w_sb[:, 1, 1], in_=w_d[:, 1, 1])
            nc.gpsimd.dma_start(out=w_sb[:, 1, 3], in_=w_d[:, 1, 3])

            # scalar queue in need-order. The 5th+ issues block the
            # scalar stream on earlier transfer completions (~15us),
            # so everything scalar needs to do before then is routed
            # to other engines.
            # tcw first: it heads the conv-start critical path (calib ->
            # scale -> fwd), the first U chunk is only needed ~2us later
            nc.scalar.dma_start(out=tcw_sb[:], in_=tcw_d[:])
            nc.scalar.dma_start(out=w_sb[:, 0, 0], in_=w_d[:, 0, 0])
            # preload the scalar engine's activation table while tcw
            # transfers (the first real ACT otherwise pays ~1.3us)
            nc.scalar.mul(pscr[0:1, 0:1, 0:1], warm_sb[0:1, 0:1], 1.0)
            nc.scalar.dma_start(out=w_sb[:, 0, 1], in_=w_d[:, 0, 1])
            # frame-0 ci1 pool on the scalar engine: runs while the first
            # w chunk transfers, without delaying the w issue chain
            nc.scalar.activation(
                pscr[:], xr0[0][:, 1].rearrange("p t h j -> p (t h) j"),
                mybir.ActivationFunctionType.Copy,
                accum_out=allxet[:, 1, 0, 2:3],
            )
            nc.scalar.dma_start(out=w_sb[:, 0, 2], in_=w_d[:, 0, 2])
            nc.scalar.dma_start(out=w_sb[:, 0, 3], in_=w_d[:, 0, 3])
            nc.scalar.dma_start(out=w_sb[:, 1, 0], in_=w_d[:, 1, 0])
            nc.scalar.dma_start(out=w_sb[:, 1, 2], in_=w_d[:, 1, 2])

            # ---- tensor: warmup matmuls (HAM stays un-throttled) ----
            for _ in range(N_WARM):
                wps = pp_conv.tile([128, H, JW], FP32, tag="conv")
                nc.tensor.matmul(
                    wps[:], lhsT=warm_sb[:, 0:128], rhs=warm_sb[:],
                    start=True, stop=True,
                )

            def pool(b, l, ci, eng="v"):
                src = xr_ap(b, l, ci).rearrange("p t h j -> p (t h) j")
                if eng == "v":
                    nc.vector.reduce_sum(
                        out=allxet[:, ci, b, 2 + l:3 + l],
                        in_=src,
                        axis=mybir.AxisListType.XY,
                    )
                else:
                    # scalar-engine pool: ACT copy with free-dim accumulate
                    nc.scalar.activation(
                        pscr[:], src,
                        mybir.ActivationFunctionType.Copy,
                        accum_out=allxet[:, ci, b, 2 + l:3 + l],
                    )

            def calib(bA, bN, l0, nl):
                # calib for batch entries bA..bA+bN-1, frames l0..l0+nl-1
                for oc in range(CC):
                    pc = pp_small.tile([128, BS, L], FP32, tag="small")
                    mms = [(ci, k) for ci in range(CC) for k in range(3)]
                    for i, (ci, k) in enumerate(mms):
                        nc.tensor.matmul(
                            pc[:, 0:bN, 0:nl],
                            lhsT=tcw_sb[:, ci, k, oc * 128:(oc + 1) * 128],
                            rhs=allxet_bf[:, ci, bA:bA + bN, k + l0:k + l0 + nl],
                            start=(i == 0),
                            stop=(i == len(mms) - 1),
                        )
                    with tc.high_priority():
                        nc.vector.tensor_scalar_add(
                            s_sb[:, oc, bA:bA + bN, l0:l0 + nl],
                            pc[:, 0:bN, 0:nl], tb1_ap(oc),
                        )

            def gate(bA, bN, l0, nl, fb_list):
                # gate conv1d for entries bA..bA+bN-1, frames l0..l0+nl-1;
                # fb (bias * (gate+1)) written only for fb_list pairs
                pg = pp_small.tile([128, BS, L], FP32, tag="small")
                mms = [(ci, k) for ci in range(CC) for k in range(3)]
                for i, (ci, k) in enumerate(mms):
                    nc.tensor.matmul(
                        pg[0:1, 0:bN, 0:nl],
                        lhsT=tcw_sb[:, ci, k, CIN:CIN + 1],
                        rhs=allxet_bf[:, ci, bA:bA + bN, k + l0:k + l0 + nl],
                        start=(i == 0),
                        stop=(i == len(mms) - 1),
                    )
                with tc.high_priority():
                    nc.vector.tensor_scalar_add(
                        g_sb[0:1, bA:bA + bN, l0:l0 + nl], pg[0:1, 0:bN, 0:nl],
                        sm_sb[0:1, CC + OC:CC + OC + 1],
                    )
                gb = pp_small.tile([128, BS, L], FP32, tag="small")
                nc.tensor.matmul(
                    gb[:, 0:bN, 0:nl], lhsT=ones_sb[0:1, :],
                    rhs=g_sb[0:1, bA:bA + bN, l0:l0 + nl],
                    start=True, stop=True,
                )
                with tc.high_priority():
                    for b, l in fb_list:
                        for oc in range(OC):
                            nc.vector.tensor_mul(
                                fb_sb[:, b, l, oc:oc + 1],
                                gb[:, b - bA, l - l0:l - l0 + 1],
                                bias_ap(oc),
                            )

            def scale_x(b, l, ci, eng="s"):
                # per-(sample, ci-chunk) channel scale folded into x,
                # writing the zero-padded 17-wide layout (even plane at
                # cols 0..15, odd plane shifted to cols 1..16); the
                # output cast produces the bf16 transform operand
                xt = x_t[(b, l, ci)]
                s_ap = s_sb[:, ci, b, l:l + 1]
                xr = xr_ap(b, l, ci)
                if eng == "v":
                    nc.vector.tensor_scalar_mul(
                        xt[:, 0:1, :, 0:JW], xr[:, 0:1], s_ap)
                    nc.vector.tensor_scalar_mul(
                        xt[:, 1:2, :, 1:JW + 1], xr[:, 1:2], s_ap)
                else:
                    nc.scalar.mul(xt[:, 0:1, :, 0:JW], xr[:, 0:1], s_ap)
                    nc.scalar.mul(xt[:, 1:2, :, 1:JW + 1], xr[:, 1:2], s_ap)

            def fwd(b, l):
                # winograd input transform for one sample (both ci),
                # xi-major so the PE's xi0 group can start after the
                # first ops. x_t layout [128, plane, h, 17] with the
                # zero pads making every op a single full-width TT.
                # V0/V1 on vector, V2/V3 on gpsimd.
                vs = []
                for ci in range(CC):
                    v = vpool.tile([128, 4, H, JW], BF16, tag="v")
                    v_t[(b, l, ci)] = v
                    vs.append((v, x_t[(b, l, ci)]))
                for v, x in vs:  # V0 = d0-d2 = odd[j]-odd[j+1]
                    nc.vector.tensor_sub(
                        out=v[:, 0], in0=x[:, 1, :, 0:JW],
                        in1=x[:, 1, :, 1:JW + 1])
                for v, x in vs:  # V1 = d1+d2 = even[j]+odd[j+1]
                    nc.vector.tensor_add(
                        out=v[:, 1], in0=x[:, 0, :, 0:JW],
                        in1=x[:, 1, :, 1:JW + 1])
                for v, x in vs:  # V2 = d2-d1
                    nc.vector.tensor_sub(
                        out=v[:, 2], in0=x[:, 1, :, 1:JW + 1],
                        in1=x[:, 0, :, 0:JW])
                for v, x in vs:  # V3 = d1-d3 = even[j]-even[j+1]
                    nc.vector.tensor_sub(
                        out=v[:, 3], in0=x[:, 0, :, 0:JW],
                        in1=x[:, 0, :, 1:JW + 1])

            # b0 l0 ci0 pool on vector (ci1 ran on scalar above); cast
            # pooled frame-0 and duplicate it into the two pre-pad window
            # columns directly in bf16. high_priority pins the casts
            # ahead of later pools in the scheduler.
            pool(0, 0, 0)
            with tc.high_priority():
                nc.vector.tensor_copy(
                    allxet_bf[:, :, 0:1, 2:3], allxet[:, :, 0:1, 2:3]
                )
                nc.vector.tensor_copy(
                    allxet_bf[:, :, 0:1, 0:1], allxet_bf[:, :, 0:1, 2:3]
                )
                nc.vector.tensor_copy(
                    allxet_bf[:, :, 0:1, 1:2], allxet_bf[:, :, 0:1, 2:3]
                )

            # frame-0 calib/gate/scale: conv (b0,l0) starts on one frame;
            # scales on vector (scalar is DMA-issue-blocked early on)
            calib(0, 1, 0, 1)
            gate(0, 1, 0, 1, [(0, 0)])
            scale_x(0, 0, 0, "v")
            scale_x(0, 0, 1, "v")
            fwd(0, 0)

            # frame 1 when it lands; the sim-side hold keeps these pools
            # from being scheduled ahead of the frame-0 chain
            with tc.tile_wait_until(0.0145):
                pool(0, 1, 0)
                pool(0, 1, 1)
            nc.vector.tensor_copy(
                allxet_bf[:, :, 0:1, 3:4], allxet[:, :, 0:1, 3:4]
            )

            def frame1_tail():
                # frame-1 calib/gate/scale/fwd as a hook inside
                # conv(0,0): its cast/PSUM latencies overlap conv
                # groups instead of delaying the first matmul
                calib(0, 1, 1, 1)
                gate(0, 1, 1, 1, [(0, 1)])
                scale_x(0, 1, 0, "s")
                scale_x(0, 1, 1, "s")
                fwd(0, 1)

            # b0 l2/l3 pools + remaining window cast (frames 3/4 on sync)
            with tc.tile_wait_until(0.018):
                pool(0, 2, 0, "v")
                pool(0, 2, 1, "s")
            with tc.tile_wait_until(0.0215):
                pool(0, 3, 0, "v")
                pool(0, 3, 1, "s")
            nc.vector.tensor_copy(
                allxet_bf[:, :, 0:1, 4:6], allxet[:, :, 0:1, 4:6]
            )

            # b1 pools track the per-frame x1 DMAs; cast + bf16-side dup
            for l in range(L):
                with tc.tile_wait_until(0.025 + 0.0035 * l):
                    pool(1, l, 0, "v")
                    pool(1, l, 1, "s")
            nc.vector.tensor_copy(
                allxet_bf[:, :, 1:2, 2:6], allxet[:, :, 1:2, 2:6]
            )
            nc.vector.tensor_copy(
                allxet_bf[:, :, 1:2, 0:1], allxet_bf[:, :, 1:2, 2:3]
            )
            nc.vector.tensor_copy(
                allxet_bf[:, :, 1:2, 1:2], allxet_bf[:, :, 1:2, 2:3]
            )

            # ---- conv groups ----
            sample_idx = [0]

            def inverse(b, l, oc, Ms, osb, h0, h1):
                # y_even = M0+M1+M2+fb ; y_odd = M1-M2-M3+fb.
                # scalar drains M0/M1/M3 (1-operand ACT affine each),
                # vector drains M2 twice (STT) and runs the two final
                # SBUF combines (fast TTs) into bf16 osb.
                fb_ap = fb_sb[:, b, l, oc:oc + 1]
                hs = slice(h0, h1)
                n = h1 - h0
                m1c = iscr.tile([128, H, JW], FP32, tag="m1c")
                a_t = iscr.tile([128, H, JW], FP32, tag="a")
                bn_t = iscr.tile([128, H, JW], FP32, tag="bn")
                e_t = iscr.tile([128, H, JW], FP32, tag="e")
                q_t = iscr.tile([128, H, JW], FP32, tag="q")
                # PSUM-freeing drains run at high priority: if they sit
                # behind bulk scale/fwd work the PE stalls on bank WARs
                with tc.high_priority():
                    nc.scalar.copy(m1c[:, 0:n], Ms[1][:, hs])
                    nc.scalar.activation(a_t[:, 0:n], Ms[0][:, hs], IDENT,
                                         bias=fb_ap, scale=1.0)
                    nc.scalar.activation(bn_t[:, 0:n], Ms[3][:, hs], IDENT,
                                         bias=fb_ap, scale=-1.0)
                    nc.vector.scalar_tensor_tensor(
                        out=e_t[:, 0:n], in0=Ms[2][:, hs], scalar=1.0,
                        in1=m1c[:, 0:n], op0=MULT, op1=ADD,
                    )
                    nc.vector.scalar_tensor_tensor(
                        out=q_t[:, 0:n], in0=Ms[2][:, hs], scalar=-1.0,
                        in1=m1c[:, 0:n], op0=MULT, op1=ADD,
                    )
                nc.vector.tensor_add(
                    out=osb[:, oc, hs, :, 0], in0=a_t[:, 0:n], in1=e_t[:, 0:n])
                nc.gpsimd.tensor_add(
                    out=osb[:, oc, hs, :, 1], in0=q_t[:, 0:n], in1=bn_t[:, 0:n])

            def conv_sample(b, l, hooks=None):
                osb = outp.tile([128, OC, H, JW, 2], BF16, tag="osb")
                gidx = 0
                last = sample_idx[0] == BS * L - 1
                for oc in range(OC):
                    Ms = []
                    for xi in range(4):
                        ps = pp_conv.tile([128, H, JW], FP32, tag="conv")
                        seq = [(ci, kh) for ci in range(CC)
                               for kh in (1, 0, 2)]
                        for i, (ci, kh) in enumerate(seq):
                            dh = kh - 1
                            hA = max(0, -dh)
                            hB = min(H, H - dh)
                            nc.tensor.matmul(
                                ps[:, hA:hB, :],
                                lhsT=w_sb[:, oc, xi, ci, kh, :],
                                rhs=v_t[(b, l, ci)][:, xi, hA + dh:hB + dh, :],
                                start=(i == 0),
                                stop=(i == len(seq) - 1),
                            )
                        Ms.append(ps)
                        gidx += 1
                        if hooks and gidx in hooks:
                            hooks[gidx]()
                    if last and oc == OC - 1:
                        # final group: h-quarter split so the drain and
                        # the stores pipeline in quarter-size chunks
                        hq = H // 4
                        for q in range(4):
                            inverse(b, l, oc, Ms, osb, q * hq, (q + 1) * hq)
                            nc.sync.dma_start(
                                out=out_d[b, l, :, oc, q * hq:(q + 1) * hq],
                                in_=osb[:, oc, q * hq:(q + 1) * hq],
                            )
                    else:
                        inverse(b, l, oc, Ms, osb, 0, H)
                        if last:
                            st_eng = nc.sync
                        else:
                            # gpsimd's software DMA is ~2x slower: it gets
                            # only the two earliest samples (sync is still
                            # streaming x in), sync carries the rest
                            st_eng = (nc.gpsimd if sample_idx[0] in (0, 2)
                                      else nc.sync)
                        st_eng.dma_start(
                            out=out_d[b, l, :, oc], in_=osb[:, oc]
                        )
                sample_idx[0] += 1

            def tail_calib():
                # b0 l2/l3 calib (frames landed during conv(0,0))
                calib(0, 1, 2, 2)
                for l in (2, 3):
                    scale_x(0, l, 0, "v")
                    scale_x(0, l, 1, "s")
                fwd(0, 2)
                fwd(0, 3)

            def tail_gate_b0():
                # frames 2/3 of b0 only — the b1 half must wait for the
                # last b1 frame DMA (~35us), which would stall the PE here
                gate(0, 1, 2, 2, [(0, 2), (0, 3)])

            def tail_gate_b1():
                gate(1, 1, 0, L, [(1, l) for l in range(L)])

            def tail_b1(l):
                # b1 calib + transforms, one frame per hook so the
                # scale/fwd bursts never starve the drain chain
                def h():
                    if l == 0:
                        calib(1, 1, 0, L)
                    scale_x(1, l, 0, "s")
                    scale_x(1, l, 1, "s")
                    fwd(1, l)
                return h

            conv_sample(0, 0, hooks={3: frame1_tail, 6: tail_calib})
            conv_sample(0, 1, hooks={6: tail_gate_b0})
            conv_sample(0, 2, hooks={2: tail_b1(0), 6: tail_gate_b1,
                                     7: tail_b1(1)})
            conv_sample(0, 3, hooks={2: tail_b1(2), 6: tail_b1(3)})
            for l in range(L):
                conv_sample(1, l)

    if _SPLIT_WAITS:
        _split_excess_waits(nc)
    return nc


def kernel(x, weight, bias, tconv_w, tconv_b, fc_w, fc_b):
    global _last_results
    x = np.asarray(x, dtype=np.float32)
    weight = np.asarray(weight, dtype=np.float32)
    bias = np.asarray(bias, dtype=np.float32)
    tconv_w = np.asarray(tconv_w, dtype=np.float32)
    tconv_b = np.asarray(tconv_b, dtype=np.float32)
    fc_w = np.asarray(fc_w, dtype=np.float32)
    fc_b = np.asarray(fc_b, dtype=np.float32)

    HW = H * W
    # host-side packing (shared across cores); 1/(H*W) pooling norm and
    # the +1 biases folded here. x parity-major: (..., 2, H, JW).
    x_bf = (x.astype(np_bf16)
            .reshape(B, L, CC, 128, H, JW, 2))
    # winograd-domain weight taps U per kh row
    wt = weight.transpose(1, 2, 3, 0)                 # (CIN, kh, kw, COUT)
    U = np.stack([
        wt[:, :, 0],
        (wt[:, :, 0] + wt[:, :, 1] + wt[:, :, 2]) * np.float32(0.5),
        (wt[:, :, 0] - wt[:, :, 1] + wt[:, :, 2]) * np.float32(0.5),
        wt[:, :, 2],
    ], axis=2)                                        # (CIN, kh, xi, COUT)
    w_host = np.ascontiguousarray(
        U.reshape(CC, 128, 3, 4, OC, 128)
        .transpose(1, 4, 3, 0, 2, 5)                  # [ci_lo,OC,xi,CC,kh,oc_lo]
        .astype(np_bf16)
    )
    inv = np.float32(1.0 / HW)
    tcw = (tconv_w * inv).transpose(1, 2, 0)          # (CIN_in, 3, CIN_out)
    fcw = (fc_w[0] * inv)[:, :, None]                 # (CIN_in, 3, 1)
    tcw_host = np.ascontiguousarray(
        np.concatenate([tcw, fcw], axis=2)
        .reshape(CC, 128, 3, CIN + 1)
        .transpose(1, 0, 2, 3)
        .astype(np_bf16)
    )
    fcw_cols = ((fc_w[0] * inv).reshape(CC, 128, 3)
                .transpose(1, 0, 2).reshape(128, CC * 3))
    sm_host = np.ascontiguousarray(np.concatenate([
        tconv_b.reshape(CC, 128).T + np.float32(1.0),
        bias.reshape(OC, 128).T,
        np.full((128, 1), fc_b[0] + 1.0, dtype=np.float32),
        fcw_cols.astype(np.float32),
    ], axis=1))

    nc = build_nc()
    in_maps = []
    for core in range(NCORES):
        xc = x_bf[core * BS:(core + 1) * BS]   # (BS, L, CC, 128, H, JW, 2)
        in_maps.append({
            # -> [L, 128, CC, 2, H, JW]
            "x0": np.ascontiguousarray(xc[0].transpose(0, 2, 1, 5, 3, 4)),
            # -> [128, L, CC, 2, H, JW]
            "x1": np.ascontiguousarray(xc[1].transpose(2, 0, 1, 5, 3, 4)),
            "w": w_host,
            "tcwfcw": tcw_host,
            "smalls": sm_host,
        })
    res = run_bass_kernel_spmd(nc, in_maps, core_ids=list(range(NCORES)))
    _last_results = res
    # out_d is [BS, L, 128, OC, H, JW, 2] partition-major bf16 ->
    # upcast + un-permute on the host
    outs = []
    for r in res.results:
        o = (np.asarray(r["out"]).astype(np.float32)
             .reshape(BS, L, 128, OC, HW).transpose(0, 1, 3, 2, 4))
        outs.append(np.ascontiguousarray(o).reshape(BS * L, COUT, H, W))
    return np.concatenate(outs, axis=0)
